# revision 15
# baseline (speedup 1.0000x reference)
"""GAT (3-layer, PyG-style) Trainium2 Bass kernel, sharded across 8 NeuronCores.

Sharding: destination-node range partition (graph parallel). Per layer each
core computes h_ext = X_own @ [W | W.a_src | W.a_dst] for its nodes,
AllGathers h_ext (split into low/high halves so the collective overlaps
compute and gather indices fit int16), then aggregates all edges whose dst is
in its range: h_ext[src] rows come in via the dma_gather ucode path, edge
softmax weights are exp(leakyrelu(al_s+al_d)) (max-subtraction skipped -
mathematically identical, fp32-safe here), and the weighted scatter-add runs
as selection-matrix matmuls accumulating numerator + denominator in PSUM.
Self-loop edges are handled analytically from local rows (no gather).

kernel(**inputs) takes the FULL inputs and returns the FULL [N, 16] output.
"""

import sys

sys.path.insert(0, "/opt/trn_rl_repo")

import numpy as np

import concourse.bass as bass
import concourse.mybir as mybir
import concourse.tile as tile
from concourse import bacc
from concourse import bass_utils
from concourse.bass_interp import get_hw_module
from concourse.masks import make_identity
from concourse import library_config

F32 = mybir.dt.float32
I16 = mybir.dt.int16
P = 128


def real_cfg():
    R = 8
    N = 50000
    PER = N // R                      # 6250 nodes per core
    T = (PER + P - 1) // P            # 49 dst tiles per core
    return dict(
        R=R, N=N, PER=PER, T=T, NPAD=T * P,
        F_IN=128, HID=64, HEADS=8, N_CLASSES=16,
        NEG=0.2, SPLIT_T=25,
    )


# ---------------------------------------------------------------------------
# Host-side preprocessing
# ---------------------------------------------------------------------------

def _wrap16(flat):
    """int16 index list -> dma_gather idx layout [128, n/16]."""
    n = flat.shape[-1]
    w = flat.reshape(flat.shape[:-1] + (n // 16, 16))      # [..., c, 16]
    w = np.swapaxes(w, -1, -2)                             # [..., 16, c]
    reps = (1,) * (flat.ndim - 1) + (8, 1)
    return np.ascontiguousarray(np.tile(w, reps), np.int16)  # [..., 128, c]


def host_prepare(inputs, cfg):
    """Build per-core in_maps (numpy). Returns (in_maps, (B_LO, B_HI))."""
    R, N, PER, T, NPAD = cfg["R"], cfg["N"], cfg["PER"], cfg["T"], cfg["NPAD"]
    F_IN, HID, HEADS, NCLS = cfg["F_IN"], cfg["HID"], cfg["HEADS"], cfg["N_CLASSES"]
    HC = HID * HEADS
    SPLIT_T = cfg["SPLIT_T"]
    LO = SPLIT_T * P
    HI = NPAD - LO

    x = np.asarray(inputs["x"], np.float32)
    ei = np.asarray(inputs["edge_index"])
    src = ei[0].astype(np.int64)
    dst = ei[1].astype(np.int64)   # self-loops handled analytically on device

    core = dst // PER
    dloc = (dst - core * PER).astype(np.int64)
    sloc = (src % PER).astype(np.int64)
    srank = (src // PER).astype(np.int64)
    is_lo = sloc < LO
    tile_of = dloc // P

    # per (core, tile, group) counts -> global max block counts
    cl = np.zeros((R, T), np.int64)
    ch = np.zeros((R, T), np.int64)
    np.add.at(cl, (core[is_lo], tile_of[is_lo]), 1)
    np.add.at(ch, (core[~is_lo], tile_of[~is_lo]), 1)
    B_LO = int(np.ceil(cl.max() / P))
    B_HI = int(np.ceil(ch.max() / P))
    B = B_LO + B_HI

    idx_lo = np.zeros((R, T, B_LO * P), np.int16)
    idx_hi = np.zeros((R, T, B_HI * P), np.int16)
    idx_dl = np.zeros((R, T, B_LO * P), np.int16)
    idx_dh = np.zeros((R, T, B_HI * P), np.int16)
    dlc = np.full((R, T, P, B), -1.0, np.float32)

    # low-group gather row ids / high-group gather row ids
    grow = np.where(is_lo, srank * LO + sloc, srank * HI + (sloc - LO))

    # order edges by (core, tile, group, anything)
    order = np.lexsort((~is_lo * 1, tile_of, core))
    g_s = grow[order]
    d_s = dloc[order]
    core_s = core[order]
    tile_s = tile_of[order]
    lo_s = is_lo[order]

    grp = core_s * (2 * T) + tile_s * 2 + (~lo_s).astype(np.int64)
    grp_start = np.searchsorted(grp, np.arange(R * T * 2), side="left")
    pos = np.arange(len(grp)) - grp_start[grp]

    lo_m = lo_s
    hi_m = ~lo_s
    idx_lo[core_s[lo_m], tile_s[lo_m], pos[lo_m]] = g_s[lo_m].astype(np.int16)
    idx_hi[core_s[hi_m], tile_s[hi_m], pos[hi_m]] = g_s[hi_m].astype(np.int16)
    idx_dl[core_s[lo_m], tile_s[lo_m], pos[lo_m]] = d_s[lo_m].astype(np.int16)
    idx_dh[core_s[hi_m], tile_s[hi_m], pos[hi_m]] = d_s[hi_m].astype(np.int16)
    # flat position within the whole tile (lo blocks then hi blocks)
    fpos = np.where(lo_m, pos, B_LO * P + pos)
    dlc[core_s, tile_s, fpos % P, fpos // P] = (d_s - tile_s * P).astype(np.float32)

    idx_lo = _wrap16(idx_lo)     # [R, T, 128, B_LO*8]
    idx_hi = _wrap16(idx_hi)
    idx_dl = _wrap16(idx_dl)
    idx_dh = _wrap16(idx_dh)

    # weight assembly: W'[f, :] = [W | W.a_src | W.a_dst | pad]
    def wext(W, a_s, a_d, ncols):
        Fin = W.shape[0]
        H, C = a_s.shape
        Wr = W.reshape(Fin, H, C)
        We = np.zeros((Fin, ncols), np.float32)
        We[:, : H * C] = W
        We[:, H * C : H * C + H] = np.einsum("fhc,hc->fh", Wr, a_s)
        We[:, H * C + H : H * C + 2 * H] = np.einsum("fhc,hc->fh", Wr, a_d)
        return We

    ROWG = 576
    ROWG2 = 64
    W0e = wext(np.asarray(inputs["W0"], np.float32),
               np.asarray(inputs["a_s0"], np.float32),
               np.asarray(inputs["a_d0"], np.float32), ROWG)
    W1e = wext(np.asarray(inputs["W1"], np.float32),
               np.asarray(inputs["a_s1"], np.float32),
               np.asarray(inputs["a_d1"], np.float32), ROWG)
    W2e = wext(np.asarray(inputs["W2"], np.float32),
               np.asarray(inputs["a_s2"], np.float32),
               np.asarray(inputs["a_d2"], np.float32), ROWG2)

    def bext(b, ncols):
        be = np.zeros((1, ncols), np.float32)
        be[0, : b.shape[0]] = b
        return np.ascontiguousarray(np.broadcast_to(be, (P, ncols)))

    b0e = bext(np.asarray(inputs["b0"], np.float32), ROWG)
    b1e = bext(np.asarray(inputs["b1"], np.float32), ROWG)
    b2e = bext(np.asarray(inputs["b2"], np.float32), ROWG2)

    W1e_r = W1e.reshape(4, P, ROWG).transpose(1, 0, 2).copy()
    W2e_r = W2e.reshape(4, P, ROWG2).transpose(1, 0, 2).copy()

    in_maps = []
    for r in range(R):
        xt0 = np.ascontiguousarray(x[r * PER : (r + 1) * PER].T)  # [F_IN, PER]
        in_maps.append({
            "xt0": xt0,
            "w0e": W0e, "w1e": W1e_r, "w2e": W2e_r,
            "b0e": b0e, "b1e": b1e, "b2e": b2e,
            "idx_lo": idx_lo[r], "idx_hi": idx_hi[r],
            "idx_dl": idx_dl[r], "idx_dh": idx_dh[r],
            "dlc": dlc[r],
        })
    return in_maps, (B_LO, B_HI)


# ---------------------------------------------------------------------------
# Device program
# ---------------------------------------------------------------------------

def build_gat_nc(cfg, BLH):
    B_LO, B_HI = BLH
    B = B_LO + B_HI
    R, PER, T, NPAD = cfg["R"], cfg["PER"], cfg["T"], cfg["NPAD"]
    F_IN, HID, HEADS, NCLS = cfg["F_IN"], cfg["HID"], cfg["HEADS"], cfg["N_CLASSES"]
    NEG = cfg["NEG"]
    HC = HID * HEADS
    ROWG = 576
    ROWG2 = 64
    SPLIT_T = cfg["SPLIT_T"]
    LO = SPLIT_T * P
    HI = NPAD - LO

    nc = bacc.Bacc("TRN2", target_bir_lowering=False, debug=False,
                   num_devices=R)

    xt0_d = nc.dram_tensor("xt0", [F_IN, PER], F32, kind="ExternalInput")
    w0e_d = nc.dram_tensor("w0e", [F_IN, ROWG], F32, kind="ExternalInput")
    w1e_d = nc.dram_tensor("w1e", [P, 4, ROWG], F32, kind="ExternalInput")
    w2e_d = nc.dram_tensor("w2e", [P, 4, ROWG2], F32, kind="ExternalInput")
    b0e_d = nc.dram_tensor("b0e", [P, ROWG], F32, kind="ExternalInput")
    b1e_d = nc.dram_tensor("b1e", [P, ROWG], F32, kind="ExternalInput")
    b2e_d = nc.dram_tensor("b2e", [P, ROWG2], F32, kind="ExternalInput")
    ilo_d = nc.dram_tensor("idx_lo", [T, P, B_LO * 8], I16, kind="ExternalInput")
    ihi_d = nc.dram_tensor("idx_hi", [T, P, B_HI * 8], I16, kind="ExternalInput")
    idl_d = nc.dram_tensor("idx_dl", [T, P, B_LO * 8], I16, kind="ExternalInput")
    idh_d = nc.dram_tensor("idx_dh", [T, P, B_HI * 8], I16, kind="ExternalInput")
    dlc_d = nc.dram_tensor("dlc", [T, P, B], F32, kind="ExternalInput")
    out_d = nc.dram_tensor("out", [PER, NCLS], F32, kind="ExternalOutput")

    rg = [list(range(R))]

    with tile.TileContext(nc) as tc:
        with (
            tc.tile_pool(name="pers", bufs=1) as pers,
            tc.tile_pool(name="sb", bufs=2) as sb,
            tc.tile_pool(name="sbS", bufs=2 * B) as sbS,
            tc.tile_pool(name="ps", bufs=2, space="PSUM") as ps,
            tc.tile_pool(name="ps1", bufs=1, space="PSUM") as ps1,
            tc.tile_pool(name="dram", bufs=1, space="DRAM") as dram,
        ):
            nc.gpsimd.load_library(library_config.mlp)

            # ---- persistent tiles ----
            Xt = pers.tile([P, 4, NPAD], F32)          # feature-major X (own nodes)
            iota_i = pers.tile([P, P], I16)
            iota_row = pers.tile([P, P], F32)
            ident = pers.tile([P, P], F32)
            nc.gpsimd.iota(iota_i[:], pattern=[[1, P]], base=0, channel_multiplier=0)
            nc.vector.tensor_copy(iota_row[:], iota_i[:])
            make_identity(nc, ident[:])

            w0_sb = pers.tile([P, 1, ROWG], F32)
            w1_sb = pers.tile([P, 4, ROWG], F32)
            w2_sb = pers.tile([P, 4, ROWG2], F32)
            b0_sb = pers.tile([P, ROWG], F32)
            b1_sb = pers.tile([P, ROWG], F32)
            b2_sb = pers.tile([P, ROWG2], F32)
            nc.sync.dma_start(w0_sb[:, 0, :], w0e_d[:, :])
            nc.sync.dma_start(w1_sb[:], w1e_d[:, :, :])
            nc.sync.dma_start(w2_sb[:], w2e_d[:, :, :])
            nc.sync.dma_start(b0_sb[:], b0e_d[:, :])
            nc.sync.dma_start(b1_sb[:], b1e_d[:, :])
            nc.sync.dma_start(b2_sb[:], b2e_d[:, :])

            if NPAD > PER:
                nc.vector.memset(Xt[:, 0, PER:NPAD], 0.0)
            nc.sync.dma_start(Xt[:, 0, :PER], xt0_d[:, :])

            # ---- internal DRAM ----
            hlo = [dram.tile([LO, ROWG], F32, name="hlo0"),
                   dram.tile([LO, ROWG], F32, name="hlo1"),
                   dram.tile([LO, ROWG2], F32, name="hlo2")]
            hhi = [dram.tile([HI, ROWG], F32, name="hhi0"),
                   dram.tile([HI, ROWG], F32, name="hhi1"),
                   dram.tile([HI, ROWG2], F32, name="hhi2")]
            tlo = [dram.tile([R * LO, ROWG], F32, addr_space="Shared", name="tlo0"),
                   dram.tile([R * LO, ROWG], F32, addr_space="Shared", name="tlo1"),
                   dram.tile([R * LO, ROWG2], F32, addr_space="Shared", name="tlo2")]
            thi = [dram.tile([R * HI, ROWG], F32, addr_space="Shared", name="thi0"),
                   dram.tile([R * HI, ROWG], F32, addr_space="Shared", name="thi1"),
                   dram.tile([R * HI, ROWG2], F32, addr_space="Shared", name="thi2")]
            aldt = [dram.tile([NPAD, 64], F32, name="ald0"),
                    dram.tile([NPAD, 64], F32, name="ald1"),
                    dram.tile([NPAD, 64], F32, name="ald2")]

            for L in range(3):
                rowg = ROWG if L < 2 else ROWG2
                KC = 1 if L == 0 else 4
                nH = HEADS if L < 2 else 1
                ncols = HC if L < 2 else NCLS
                W_sb = [w0_sb, w1_sb, w2_sb][L]
                b_sb = [b0_sb, b1_sb, b2_sb][L]
                alow = ncols
                adoff = ncols + nH

                # ---------- h_ext = X_own @ W' + b' ----------
                for nt in range(T):
                    ph = ps1.tile([P, max(rowg, 528)], F32, tag="ph")
                    n1 = min(512, rowg)
                    for kc in range(KC):
                        nc.tensor.matmul(
                            ph[:, 0:n1],
                            lhsT=Xt[:, kc, nt * P : (nt + 1) * P],
                            rhs=W_sb[:, kc, 0:n1],
                            start=(kc == 0), stop=(kc == KC - 1),
                        )
                    if rowg > 512:
                        for kc in range(KC):
                            nc.tensor.matmul(
                                ph[:, 512:rowg],
                                lhsT=Xt[:, kc, nt * P : (nt + 1) * P],
                                rhs=W_sb[:, kc, 512:rowg],
                                start=(kc == 0), stop=(kc == KC - 1),
                            )
                    hsb = sb.tile([P, rowg], F32, tag="hsb")
                    nc.vector.tensor_tensor(hsb[:], ph[:, 0:rowg], b_sb[:],
                                            mybir.AluOpType.add)
                    if nt < SPLIT_T:
                        nc.sync.dma_start(hlo[L][nt * P : (nt + 1) * P, :], hsb[:])
                    else:
                        r0 = nt * P - LO
                        nc.sync.dma_start(hhi[L][r0 : r0 + P, :], hsb[:])
                    nc.sync.dma_start(aldt[L][nt * P : (nt + 1) * P, 0:nH],
                                      hsb[:, adoff : adoff + nH])
                    if nt == SPLIT_T - 1:
                        nc.gpsimd.collective_compute(
                            "AllGather", mybir.AluOpType.bypass,
                            replica_groups=rg, ins=[hlo[L][:, :]],
                            outs=[tlo[L][:, :]])
                nc.gpsimd.collective_compute(
                    "AllGather", mybir.AluOpType.bypass,
                    replica_groups=rg, ins=[hhi[L][:, :]],
                    outs=[thi[L][:, :]])

                # ---------- edge aggregation per dst tile ----------
                for t in range(T):
                    ilo = sb.tile([P, B_LO * 8], I16, tag="ilo")
                    ihi = sb.tile([P, B_HI * 8], I16, tag="ihi")
                    idl = sb.tile([P, B_LO * 8], I16, tag="idl")
                    idh = sb.tile([P, B_HI * 8], I16, tag="idh")
                    dlct = sb.tile([P, B], F32, tag="dlct")
                    nc.sync.dma_start(ilo[:], ilo_d[t, :, :])
                    nc.sync.dma_start(ihi[:], ihi_d[t, :, :])
                    nc.sync.dma_start(idl[:], idl_d[t, :, :])
                    nc.sync.dma_start(idh[:], idh_d[t, :, :])
                    nc.sync.dma_start(dlct[:], dlc_d[t, :, :])

                    # local rows for the analytic self-loop term
                    loc = sb.tile([P, rowg], F32, tag="loc")
                    if t < SPLIT_T:
                        nc.sync.dma_start(loc[:], hlo[L][t * P : (t + 1) * P, :])
                    else:
                        r0 = t * P - LO
                        nc.sync.dma_start(loc[:], hhi[L][r0 : r0 + P, :])

                    g = sb.tile([P, B, rowg], F32, tag="g")
                    nc.gpsimd.dma_gather(
                        g[:, 0:B_LO, :], tlo[L][:, :], ilo[:],
                        num_idxs=B_LO * P, num_idxs_reg=B_LO * P,
                        elem_size=rowg)
                    nc.gpsimd.dma_gather(
                        g[:, B_LO:B, :], thi[L][:, :], ihi[:],
                        num_idxs=B_HI * P, num_idxs_reg=B_HI * P,
                        elem_size=rowg)
                    adem = sb.tile([P, B, 64], F32, tag="adem")
                    nc.gpsimd.dma_gather(
                        adem[:, 0:B_LO, :], aldt[L][:, :], idl[:],
                        num_idxs=B_LO * P, num_idxs_reg=B_LO * P,
                        elem_size=64)
                    nc.gpsimd.dma_gather(
                        adem[:, B_LO:B, :], aldt[L][:, :], idh[:],
                        num_idxs=B_HI * P, num_idxs_reg=B_HI * P,
                        elem_size=64)

                    # logits = al_s[src] + al_d[dst] ; lrelu ; exp
                    logits = sb.tile([P, B * nH], F32, tag="logits")
                    nc.vector.tensor_tensor(
                        logits[:].rearrange("p (b h) -> p b h", b=B),
                        g[:, :, alow : alow + nH],
                        adem[:, :, 0:nH],
                        mybir.AluOpType.add)
                    lr = sb.tile([P, B * nH], F32, tag="lr")
                    nc.vector.tensor_scalar_mul(lr[:], logits[:], NEG)
                    nc.vector.tensor_tensor(lr[:], lr[:], logits[:],
                                            mybir.AluOpType.max)
                    w = sb.tile([P, B * nH], F32, tag="w")
                    nc.scalar.activation(w[:], lr[:],
                                         mybir.ActivationFunctionType.Exp)

                    # self-loop: ws = exp(lrelu(al_s_loc + al_d_loc))
                    sl = sb.tile([P, 3 * nH], F32, tag="sl")
                    nc.vector.tensor_tensor(sl[:, 0:nH],
                                            loc[:, alow : alow + nH],
                                            loc[:, adoff : adoff + nH],
                                            mybir.AluOpType.add)
                    nc.vector.tensor_scalar_mul(sl[:, nH : 2 * nH], sl[:, 0:nH], NEG)
                    nc.vector.tensor_tensor(sl[:, nH : 2 * nH], sl[:, nH : 2 * nH],
                                            sl[:, 0:nH], mybir.AluOpType.max)
                    ws = sb.tile([P, nH], F32, tag="ws")
                    nc.scalar.activation(ws[:], sl[:, nH : 2 * nH],
                                         mybir.ActivationFunctionType.Exp)

                    # weight gathered rows in place (al columns untouched)
                    nc.vector.tensor_tensor(
                        g[:, :, 0:ncols].rearrange("p b (h c) -> p b h c", h=nH),
                        g[:, :, 0:ncols].rearrange("p b (h c) -> p b h c", h=nH),
                        w[:].rearrange("p (b h) -> p b h", b=B)
                            .unsqueeze(3)
                            .to_broadcast([P, B, nH, ncols // nH]),
                        mybir.AluOpType.mult)

                    po = ps.tile([P, ncols], F32, tag="po")
                    pd = ps.tile([P, nH], F32, tag="pd")
                    for b in range(B):
                        S_b = sbS.tile([P, P], F32, tag="S")
                        nc.vector.tensor_tensor(
                            S_b[:],
                            dlct[:, b : b + 1].to_broadcast([P, P]),
                            iota_row[:],
                            mybir.AluOpType.is_equal)
                        nc.tensor.matmul(po[:], lhsT=S_b[:], rhs=g[:, b, 0:ncols],
                                         start=(b == 0), stop=(b == B - 1))
                        nc.tensor.matmul(pd[:], lhsT=S_b[:],
                                         rhs=w[:, b * nH : (b + 1) * nH],
                                         start=(b == 0), stop=(b == B - 1))

                    den = sb.tile([P, nH], F32, tag="den")
                    nc.vector.tensor_tensor(den[:], pd[:], ws[:],
                                            mybir.AluOpType.add)
                    rden = sb.tile([P, nH], F32, tag="rden")
                    nc.vector.reciprocal(rden[:], den[:])
                    wr = sb.tile([P, nH], F32, tag="wr")
                    nc.vector.tensor_tensor(wr[:], ws[:], rden[:],
                                            mybir.AluOpType.mult)

                    xn = sb.tile([P, ncols], F32, tag="xn")
                    nc.vector.tensor_tensor(
                        xn[:].rearrange("p (h c) -> p h c", h=nH),
                        po[:].rearrange("p (h c) -> p h c", h=nH),
                        rden[:].unsqueeze(2).to_broadcast([P, nH, ncols // nH]),
                        mybir.AluOpType.mult)
                    t2 = sb.tile([P, ncols], F32, tag="t2")
                    nc.vector.tensor_tensor(
                        t2[:].rearrange("p (h c) -> p h c", h=nH),
                        loc[:, 0:ncols].rearrange("p (h c) -> p h c", h=nH),
                        wr[:].unsqueeze(2).to_broadcast([P, nH, ncols // nH]),
                        mybir.AluOpType.mult)
                    nc.vector.tensor_tensor(xn[:], xn[:], t2[:],
                                            mybir.AluOpType.add)

                    if L < 2:
                        # ELU: xe = relu(x) + exp(min(x,0)) - 1
                        m = sb.tile([P, ncols], F32, tag="t2")
                        nc.vector.tensor_scalar_min(m[:], xn[:], 0.0)
                        em = sb.tile([P, ncols], F32, tag="t2")
                        nc.scalar.activation(em[:], m[:],
                                             mybir.ActivationFunctionType.Exp)
                        xe = sb.tile([P, ncols], F32, tag="xe")
                        nc.vector.tensor_scalar_max(xe[:], xn[:], 0.0)
                        nc.vector.tensor_tensor(xe[:], xe[:], em[:],
                                                mybir.AluOpType.add)
                        nc.vector.tensor_scalar_add(xe[:], xe[:], -1.0)
                        for c4 in range(ncols // P):
                            pt = ps.tile([P, P], F32, tag="pt")
                            nc.tensor.transpose(
                                pt[:], xe[:, c4 * P : (c4 + 1) * P], ident[:])
                            nc.scalar.copy(Xt[:, c4, t * P : (t + 1) * P], pt[:])
                    else:
                        rows = min(P, PER - t * P)
                        nc.sync.dma_start(out_d[t * P : t * P + rows, :],
                                          xn[:rows, 0:NCLS])

    nc.compile()
    nc.m = get_hw_module(nc.m)
    return nc


# ---------------------------------------------------------------------------
# Entry point
# ---------------------------------------------------------------------------

_CACHE = {}


def _get_nc(cfg, BLH):
    key = (tuple(sorted(cfg.items())), BLH)
    if key not in _CACHE:
        _CACHE[key] = build_gat_nc(cfg, BLH)
    return _CACHE[key]


def run(inputs, trace=False):
    cfg = real_cfg()
    in_maps, BLH = host_prepare(inputs, cfg)
    nc = _get_nc(cfg, BLH)
    res = bass_utils.run_bass_kernel_spmd(
        nc, in_maps, core_ids=list(range(cfg["R"])), trace=trace)
    out = np.concatenate([res.results[r]["out"] for r in range(cfg["R"])], axis=0)
    return out[: cfg["N"]], res


def kernel(**inputs) -> np.ndarray:
    out, _ = run(inputs, trace=False)
    return out.astype(np.float32)


# revision 16
# speedup vs baseline: 1.2236x; 1.2236x over previous
"""GAT (3-layer, PyG-style) Trainium2 Bass kernel, sharded across 8 NeuronCores.

Sharding: destination-node range partition (graph parallel). Per layer each
core computes h_ext = X_own @ [W | W.a_src | W.a_dst] for its nodes,
AllGathers h_ext (split into low/high halves so the collective overlaps
compute and gather indices fit int16), then aggregates all edges whose dst is
in its range: h_ext[src] rows come in via the dma_gather ucode path, edge
softmax weights are exp(leakyrelu(al_s+al_d)) (max-subtraction skipped -
mathematically identical, fp32-safe here), and the weighted scatter-add runs
as selection-matrix matmuls accumulating numerator + denominator in PSUM.
Self-loop edges are handled analytically from local rows (no gather).

kernel(**inputs) takes the FULL inputs and returns the FULL [N, 16] output.
"""

import sys

sys.path.insert(0, "/opt/trn_rl_repo")

import numpy as np

import concourse.bass as bass
import concourse.mybir as mybir
import concourse.tile as tile
from concourse import bacc
from concourse import bass_utils
from concourse.bass_interp import get_hw_module
from concourse.masks import make_identity
from concourse import library_config

F32 = mybir.dt.float32
BF = mybir.dt.bfloat16
I16 = mybir.dt.int16
import ml_dtypes
NPBF = ml_dtypes.bfloat16
P = 128


def real_cfg():
    R = 8
    N = 50000
    PER = N // R                      # 6250 nodes per core
    T = (PER + P - 1) // P            # 49 dst tiles per core
    return dict(
        R=R, N=N, PER=PER, T=T, NPAD=T * P,
        F_IN=128, HID=64, HEADS=8, N_CLASSES=16,
        NEG=0.2, SPLIT_T=25,
    )


# ---------------------------------------------------------------------------
# Host-side preprocessing
# ---------------------------------------------------------------------------

def _wrap16(flat):
    """int16 index list -> dma_gather idx layout [128, n/16]."""
    n = flat.shape[-1]
    w = flat.reshape(flat.shape[:-1] + (n // 16, 16))      # [..., c, 16]
    w = np.swapaxes(w, -1, -2)                             # [..., 16, c]
    reps = (1,) * (flat.ndim - 1) + (8, 1)
    return np.ascontiguousarray(np.tile(w, reps), np.int16)  # [..., 128, c]


def host_prepare(inputs, cfg):
    """Build per-core in_maps (numpy). Returns (in_maps, (B_LO, B_HI))."""
    R, N, PER, T, NPAD = cfg["R"], cfg["N"], cfg["PER"], cfg["T"], cfg["NPAD"]
    F_IN, HID, HEADS, NCLS = cfg["F_IN"], cfg["HID"], cfg["HEADS"], cfg["N_CLASSES"]
    HC = HID * HEADS
    SPLIT_T = cfg["SPLIT_T"]
    LO = SPLIT_T * P
    HI = NPAD - LO

    x = np.asarray(inputs["x"], np.float32)
    ei = np.asarray(inputs["edge_index"])
    src = ei[0].astype(np.int64)
    dst = ei[1].astype(np.int64)   # self-loops handled analytically on device

    core = dst // PER
    dloc = (dst - core * PER).astype(np.int64)
    sloc = (src % PER).astype(np.int64)
    srank = (src // PER).astype(np.int64)
    is_lo = sloc < LO
    tile_of = dloc // P

    # per (core, tile, group) counts -> global max block counts
    cl = np.zeros((R, T), np.int64)
    ch = np.zeros((R, T), np.int64)
    np.add.at(cl, (core[is_lo], tile_of[is_lo]), 1)
    np.add.at(ch, (core[~is_lo], tile_of[~is_lo]), 1)
    B_LO = int(np.ceil(cl.max() / P))
    B_HI = int(np.ceil(ch.max() / P))
    B = B_LO + B_HI

    idx_lo = np.zeros((R, T, B_LO * P), np.int16)
    idx_hi = np.zeros((R, T, B_HI * P), np.int16)
    idx_dl = np.zeros((R, T, B_LO * P), np.int16)
    idx_dh = np.zeros((R, T, B_HI * P), np.int16)
    dlc = np.full((R, T, P, B), -1.0, np.float32)  # cast to bf16 at the end

    # low-group gather row ids / high-group gather row ids
    grow = np.where(is_lo, srank * LO + sloc, srank * HI + (sloc - LO))

    # order edges by (core, tile, group, anything)
    order = np.lexsort((~is_lo * 1, tile_of, core))
    g_s = grow[order]
    d_s = dloc[order]
    core_s = core[order]
    tile_s = tile_of[order]
    lo_s = is_lo[order]

    grp = core_s * (2 * T) + tile_s * 2 + (~lo_s).astype(np.int64)
    grp_start = np.searchsorted(grp, np.arange(R * T * 2), side="left")
    pos = np.arange(len(grp)) - grp_start[grp]

    lo_m = lo_s
    hi_m = ~lo_s
    idx_lo[core_s[lo_m], tile_s[lo_m], pos[lo_m]] = g_s[lo_m].astype(np.int16)
    idx_hi[core_s[hi_m], tile_s[hi_m], pos[hi_m]] = g_s[hi_m].astype(np.int16)
    idx_dl[core_s[lo_m], tile_s[lo_m], pos[lo_m]] = d_s[lo_m].astype(np.int16)
    idx_dh[core_s[hi_m], tile_s[hi_m], pos[hi_m]] = d_s[hi_m].astype(np.int16)
    # flat position within the whole tile (lo blocks then hi blocks)
    fpos = np.where(lo_m, pos, B_LO * P + pos)
    dlc[core_s, tile_s, fpos % P, fpos // P] = (d_s - tile_s * P).astype(np.float32)
    dlc = dlc.astype(NPBF)

    idx_lo = _wrap16(idx_lo)     # [R, T, 128, B_LO*8]
    idx_hi = _wrap16(idx_hi)
    idx_dl = _wrap16(idx_dl)
    idx_dh = _wrap16(idx_dh)

    # weight assembly: W'[f, :] = [W | W.a_src | W.a_dst | pad]
    def wext(W, a_s, a_d, ncols):
        Fin = W.shape[0]
        H, C = a_s.shape
        Wr = W.reshape(Fin, H, C)
        We = np.zeros((Fin, ncols), np.float32)
        We[:, : H * C] = W
        We[:, H * C : H * C + H] = np.einsum("fhc,hc->fh", Wr, a_s)
        We[:, H * C + H : H * C + 2 * H] = np.einsum("fhc,hc->fh", Wr, a_d)
        return We

    ROWG = 640
    ROWG2 = 128
    W0e = wext(np.asarray(inputs["W0"], np.float32),
               np.asarray(inputs["a_s0"], np.float32),
               np.asarray(inputs["a_d0"], np.float32), ROWG)
    W1e = wext(np.asarray(inputs["W1"], np.float32),
               np.asarray(inputs["a_s1"], np.float32),
               np.asarray(inputs["a_d1"], np.float32), ROWG)
    W2e = wext(np.asarray(inputs["W2"], np.float32),
               np.asarray(inputs["a_s2"], np.float32),
               np.asarray(inputs["a_d2"], np.float32), ROWG2)

    def bext(b, ncols):
        be = np.zeros((1, ncols), np.float32)
        be[0, : b.shape[0]] = b
        return np.ascontiguousarray(np.broadcast_to(be, (P, ncols)))

    b0e = bext(np.asarray(inputs["b0"], np.float32), ROWG)
    b1e = bext(np.asarray(inputs["b1"], np.float32), ROWG)
    b2e = bext(np.asarray(inputs["b2"], np.float32), ROWG2)

    W1e_r = W1e.reshape(4, P, ROWG).transpose(1, 0, 2).copy()
    W2e_r = W2e.reshape(4, P, ROWG2).transpose(1, 0, 2).copy()

    in_maps = []
    for r in range(R):
        xt0 = np.ascontiguousarray(x[r * PER : (r + 1) * PER].T)  # [F_IN, PER]
        in_maps.append({
            "xt0": xt0,
            "w0e": W0e, "w1e": W1e_r, "w2e": W2e_r,
            "b0e": b0e, "b1e": b1e, "b2e": b2e,
            "idx_lo": idx_lo[r], "idx_hi": idx_hi[r],
            "idx_dl": idx_dl[r], "idx_dh": idx_dh[r],
            "dlc": dlc[r],
        })
    return in_maps, (B_LO, B_HI)


# ---------------------------------------------------------------------------
# Device program
# ---------------------------------------------------------------------------

def build_gat_nc(cfg, BLH):
    B_LO, B_HI = BLH
    B = B_LO + B_HI
    R, PER, T, NPAD = cfg["R"], cfg["PER"], cfg["T"], cfg["NPAD"]
    F_IN, HID, HEADS, NCLS = cfg["F_IN"], cfg["HID"], cfg["HEADS"], cfg["N_CLASSES"]
    NEG = cfg["NEG"]
    HC = HID * HEADS
    ROWG = 640
    ROWG2 = 128
    SPLIT_T = cfg["SPLIT_T"]
    LO = SPLIT_T * P
    HI = NPAD - LO

    nc = bacc.Bacc("TRN2", target_bir_lowering=False, debug=False,
                   num_devices=R)

    xt0_d = nc.dram_tensor("xt0", [F_IN, PER], F32, kind="ExternalInput")
    w0e_d = nc.dram_tensor("w0e", [F_IN, ROWG], F32, kind="ExternalInput")
    w1e_d = nc.dram_tensor("w1e", [P, 4, ROWG], F32, kind="ExternalInput")
    w2e_d = nc.dram_tensor("w2e", [P, 4, ROWG2], F32, kind="ExternalInput")
    b0e_d = nc.dram_tensor("b0e", [P, ROWG], F32, kind="ExternalInput")
    b1e_d = nc.dram_tensor("b1e", [P, ROWG], F32, kind="ExternalInput")
    b2e_d = nc.dram_tensor("b2e", [P, ROWG2], F32, kind="ExternalInput")
    ilo_d = nc.dram_tensor("idx_lo", [T, P, B_LO * 8], I16, kind="ExternalInput")
    ihi_d = nc.dram_tensor("idx_hi", [T, P, B_HI * 8], I16, kind="ExternalInput")
    idl_d = nc.dram_tensor("idx_dl", [T, P, B_LO * 8], I16, kind="ExternalInput")
    idh_d = nc.dram_tensor("idx_dh", [T, P, B_HI * 8], I16, kind="ExternalInput")
    dlc_d = nc.dram_tensor("dlc", [T, P, B], BF, kind="ExternalInput")
    out_d = nc.dram_tensor("out", [PER, NCLS], F32, kind="ExternalOutput")

    rg = [list(range(R))]

    with tile.TileContext(nc) as tc:
        with (
            tc.tile_pool(name="pers", bufs=1) as pers,
            tc.tile_pool(name="sb", bufs=2) as sb,
            tc.tile_pool(name="sbS", bufs=2 * B) as sbS,
            tc.tile_pool(name="ps", bufs=2, space="PSUM") as ps,
            tc.tile_pool(name="ps1", bufs=1, space="PSUM") as ps1,
            tc.tile_pool(name="dram", bufs=1, space="DRAM") as dram,
        ):
            nc.gpsimd.load_library(library_config.mlp)

            # ---- persistent tiles ----
            Xt = pers.tile([P, 4, NPAD], F32)          # feature-major X (own nodes)
            iota_i = pers.tile([P, P], I16)
            iota_row = pers.tile([P, P], BF)
            ident = pers.tile([P, P], F32)
            nc.gpsimd.iota(iota_i[:], pattern=[[1, P]], base=0, channel_multiplier=0)
            nc.vector.tensor_copy(iota_row[:], iota_i[:])
            make_identity(nc, ident[:])

            w0_sb = pers.tile([P, 1, ROWG], F32)
            w1_sb = pers.tile([P, 4, ROWG], F32)
            w2_sb = pers.tile([P, 4, ROWG2], F32)
            b0_sb = pers.tile([P, ROWG], F32)
            b1_sb = pers.tile([P, ROWG], F32)
            b2_sb = pers.tile([P, ROWG2], F32)
            nc.sync.dma_start(w0_sb[:, 0, :], w0e_d[:, :])
            nc.sync.dma_start(w1_sb[:], w1e_d[:, :, :])
            nc.sync.dma_start(w2_sb[:], w2e_d[:, :, :])
            nc.sync.dma_start(b0_sb[:], b0e_d[:, :])
            nc.sync.dma_start(b1_sb[:], b1e_d[:, :])
            nc.sync.dma_start(b2_sb[:], b2e_d[:, :])

            if NPAD > PER:
                nc.vector.memset(Xt[:, 0, PER:NPAD], 0.0)
            nc.sync.dma_start(Xt[:, 0, :PER], xt0_d[:, :])

            # ---- internal DRAM ----
            hlo = [dram.tile([LO, ROWG], BF, name="hlo0"),
                   dram.tile([LO, ROWG], BF, name="hlo1"),
                   dram.tile([LO, ROWG2], BF, name="hlo2")]
            hhi = [dram.tile([HI, ROWG], BF, name="hhi0"),
                   dram.tile([HI, ROWG], BF, name="hhi1"),
                   dram.tile([HI, ROWG2], BF, name="hhi2")]
            tlo = [dram.tile([R * LO, ROWG], BF, addr_space="Shared", name="tlo0"),
                   dram.tile([R * LO, ROWG], BF, addr_space="Shared", name="tlo1"),
                   dram.tile([R * LO, ROWG2], BF, addr_space="Shared", name="tlo2")]
            thi = [dram.tile([R * HI, ROWG], BF, addr_space="Shared", name="thi0"),
                   dram.tile([R * HI, ROWG], BF, addr_space="Shared", name="thi1"),
                   dram.tile([R * HI, ROWG2], BF, addr_space="Shared", name="thi2")]
            aldt = [dram.tile([NPAD, 128], BF, name="ald0"),
                    dram.tile([NPAD, 128], BF, name="ald1"),
                    dram.tile([NPAD, 128], BF, name="ald2")]

            for L in range(3):
                rowg = ROWG if L < 2 else ROWG2
                KC = 1 if L == 0 else 4
                nH = HEADS if L < 2 else 1
                ncols = HC if L < 2 else NCLS
                W_sb = [w0_sb, w1_sb, w2_sb][L]
                b_sb = [b0_sb, b1_sb, b2_sb][L]
                alow = ncols
                adoff = ncols + nH

                # ---------- h_ext = X_own @ W' + b' ----------
                for nt in range(T):
                    ph = ps1.tile([P, max(rowg, 528)], F32, tag="ph")
                    n1 = min(512, rowg)
                    for kc in range(KC):
                        nc.tensor.matmul(
                            ph[:, 0:n1],
                            lhsT=Xt[:, kc, nt * P : (nt + 1) * P],
                            rhs=W_sb[:, kc, 0:n1],
                            start=(kc == 0), stop=(kc == KC - 1),
                        )
                    if rowg > 512:
                        for kc in range(KC):
                            nc.tensor.matmul(
                                ph[:, 512:rowg],
                                lhsT=Xt[:, kc, nt * P : (nt + 1) * P],
                                rhs=W_sb[:, kc, 512:rowg],
                                start=(kc == 0), stop=(kc == KC - 1),
                            )
                    hsb = sb.tile([P, rowg], BF, tag="hsb")
                    nc.vector.tensor_tensor(hsb[:], ph[:, 0:rowg], b_sb[:],
                                            mybir.AluOpType.add)
                    if nt < SPLIT_T:
                        nc.sync.dma_start(hlo[L][nt * P : (nt + 1) * P, :], hsb[:])
                    else:
                        r0 = nt * P - LO
                        nc.sync.dma_start(hhi[L][r0 : r0 + P, :], hsb[:])
                    nc.sync.dma_start(aldt[L][nt * P : (nt + 1) * P, 0:nH],
                                      hsb[:, adoff : adoff + nH])
                    if nt == SPLIT_T - 1:
                        nc.gpsimd.collective_compute(
                            "AllGather", mybir.AluOpType.bypass,
                            replica_groups=rg, ins=[hlo[L][:, :]],
                            outs=[tlo[L][:, :]])
                nc.gpsimd.collective_compute(
                    "AllGather", mybir.AluOpType.bypass,
                    replica_groups=rg, ins=[hhi[L][:, :]],
                    outs=[thi[L][:, :]])

                # ---------- edge aggregation per dst tile ----------
                for t in range(T):
                    ilo = sb.tile([P, B_LO * 8], I16, tag="ilo")
                    ihi = sb.tile([P, B_HI * 8], I16, tag="ihi")
                    idl = sb.tile([P, B_LO * 8], I16, tag="idl")
                    idh = sb.tile([P, B_HI * 8], I16, tag="idh")
                    dlct = sb.tile([P, B], BF, tag="dlct")
                    nc.sync.dma_start(ilo[:], ilo_d[t, :, :])
                    nc.sync.dma_start(ihi[:], ihi_d[t, :, :])
                    nc.sync.dma_start(idl[:], idl_d[t, :, :])
                    nc.sync.dma_start(idh[:], idh_d[t, :, :])
                    nc.sync.dma_start(dlct[:], dlc_d[t, :, :])

                    # local rows for the analytic self-loop term
                    loc = sb.tile([P, rowg], BF, tag="loc")
                    if t < SPLIT_T:
                        nc.sync.dma_start(loc[:], hlo[L][t * P : (t + 1) * P, :])
                    else:
                        r0 = t * P - LO
                        nc.sync.dma_start(loc[:], hhi[L][r0 : r0 + P, :])

                    g = sb.tile([P, B, rowg], BF, tag="g")
                    nc.gpsimd.dma_gather(
                        g[:, 0:B_LO, :], tlo[L][:, :], ilo[:],
                        num_idxs=B_LO * P, num_idxs_reg=B_LO * P,
                        elem_size=rowg)
                    nc.gpsimd.dma_gather(
                        g[:, B_LO:B, :], thi[L][:, :], ihi[:],
                        num_idxs=B_HI * P, num_idxs_reg=B_HI * P,
                        elem_size=rowg)
                    adem = sb.tile([P, B, 128], BF, tag="adem")
                    nc.gpsimd.dma_gather(
                        adem[:, 0:B_LO, :], aldt[L][:, :], idl[:],
                        num_idxs=B_LO * P, num_idxs_reg=B_LO * P,
                        elem_size=128)
                    nc.gpsimd.dma_gather(
                        adem[:, B_LO:B, :], aldt[L][:, :], idh[:],
                        num_idxs=B_HI * P, num_idxs_reg=B_HI * P,
                        elem_size=128)

                    # logits = al_s[src] + al_d[dst] ; lrelu ; exp
                    logits = sb.tile([P, B * nH], F32, tag="logits")
                    nc.vector.tensor_tensor(
                        logits[:].rearrange("p (b h) -> p b h", b=B),
                        g[:, :, alow : alow + nH],
                        adem[:, :, 0:nH],
                        mybir.AluOpType.add)
                    lr = sb.tile([P, B * nH], F32, tag="lr")
                    nc.vector.tensor_scalar_mul(lr[:], logits[:], NEG)
                    nc.vector.tensor_tensor(lr[:], lr[:], logits[:],
                                            mybir.AluOpType.max)
                    w = sb.tile([P, B * nH], BF, tag="w")
                    nc.scalar.activation(w[:], lr[:],
                                         mybir.ActivationFunctionType.Exp)

                    # self-loop: ws = exp(lrelu(al_s_loc + al_d_loc))
                    sl = sb.tile([P, 3 * nH], F32, tag="sl")
                    nc.vector.tensor_tensor(sl[:, 0:nH],
                                            loc[:, alow : alow + nH],
                                            loc[:, adoff : adoff + nH],
                                            mybir.AluOpType.add)
                    nc.vector.tensor_scalar_mul(sl[:, nH : 2 * nH], sl[:, 0:nH], NEG)
                    nc.vector.tensor_tensor(sl[:, nH : 2 * nH], sl[:, nH : 2 * nH],
                                            sl[:, 0:nH], mybir.AluOpType.max)
                    ws = sb.tile([P, nH], F32, tag="ws")
                    nc.scalar.activation(ws[:], sl[:, nH : 2 * nH],
                                         mybir.ActivationFunctionType.Exp)

                    # weight gathered rows in place (al columns untouched)
                    nc.vector.tensor_tensor(
                        g[:, :, 0:ncols].rearrange("p b (h c) -> p b h c", h=nH),
                        g[:, :, 0:ncols].rearrange("p b (h c) -> p b h c", h=nH),
                        w[:].rearrange("p (b h) -> p b h", b=B)
                            .unsqueeze(3)
                            .to_broadcast([P, B, nH, ncols // nH]),
                        mybir.AluOpType.mult)

                    po = ps.tile([P, ncols], F32, tag="po")
                    pd = ps.tile([P, nH], F32, tag="pd")
                    for b in range(B):
                        S_b = sbS.tile([P, P], BF, tag="S")
                        nc.vector.tensor_tensor(
                            S_b[:],
                            dlct[:, b : b + 1].to_broadcast([P, P]),
                            iota_row[:],
                            mybir.AluOpType.is_equal)
                        nc.tensor.matmul(po[:], lhsT=S_b[:], rhs=g[:, b, 0:ncols],
                                         start=(b == 0), stop=(b == B - 1))
                        nc.tensor.matmul(pd[:], lhsT=S_b[:],
                                         rhs=w[:, b * nH : (b + 1) * nH],
                                         start=(b == 0), stop=(b == B - 1))

                    den = sb.tile([P, nH], F32, tag="den")
                    nc.vector.tensor_tensor(den[:], pd[:], ws[:],
                                            mybir.AluOpType.add)
                    rden = sb.tile([P, nH], F32, tag="rden")
                    nc.vector.reciprocal(rden[:], den[:])
                    wr = sb.tile([P, nH], F32, tag="wr")
                    nc.vector.tensor_tensor(wr[:], ws[:], rden[:],
                                            mybir.AluOpType.mult)

                    xn = sb.tile([P, ncols], F32, tag="xn")
                    nc.vector.tensor_tensor(
                        xn[:].rearrange("p (h c) -> p h c", h=nH),
                        po[:].rearrange("p (h c) -> p h c", h=nH),
                        rden[:].unsqueeze(2).to_broadcast([P, nH, ncols // nH]),
                        mybir.AluOpType.mult)
                    t2 = sb.tile([P, ncols], F32, tag="t2")
                    nc.vector.tensor_tensor(
                        t2[:].rearrange("p (h c) -> p h c", h=nH),
                        loc[:, 0:ncols].rearrange("p (h c) -> p h c", h=nH),
                        wr[:].unsqueeze(2).to_broadcast([P, nH, ncols // nH]),
                        mybir.AluOpType.mult)
                    nc.vector.tensor_tensor(xn[:], xn[:], t2[:],
                                            mybir.AluOpType.add)

                    if L < 2:
                        # ELU: xe = relu(x) + exp(min(x,0)) - 1
                        m = sb.tile([P, ncols], F32, tag="t2")
                        nc.vector.tensor_scalar_min(m[:], xn[:], 0.0)
                        em = sb.tile([P, ncols], F32, tag="t2")
                        nc.scalar.activation(em[:], m[:],
                                             mybir.ActivationFunctionType.Exp)
                        xe = sb.tile([P, ncols], F32, tag="xe")
                        nc.vector.tensor_scalar_max(xe[:], xn[:], 0.0)
                        nc.vector.tensor_tensor(xe[:], xe[:], em[:],
                                                mybir.AluOpType.add)
                        nc.vector.tensor_scalar_add(xe[:], xe[:], -1.0)
                        for c4 in range(ncols // P):
                            pt = ps.tile([P, P], F32, tag="pt")
                            nc.tensor.transpose(
                                pt[:], xe[:, c4 * P : (c4 + 1) * P], ident[:])
                            nc.scalar.copy(Xt[:, c4, t * P : (t + 1) * P], pt[:])
                    else:
                        rows = min(P, PER - t * P)
                        nc.sync.dma_start(out_d[t * P : t * P + rows, :],
                                          xn[:rows, 0:NCLS])

    nc.compile()
    nc.m = get_hw_module(nc.m)
    return nc


# ---------------------------------------------------------------------------
# Entry point
# ---------------------------------------------------------------------------

_CACHE = {}


def _get_nc(cfg, BLH):
    key = (tuple(sorted(cfg.items())), BLH)
    if key not in _CACHE:
        _CACHE[key] = build_gat_nc(cfg, BLH)
    return _CACHE[key]


def run(inputs, trace=False):
    cfg = real_cfg()
    in_maps, BLH = host_prepare(inputs, cfg)
    nc = _get_nc(cfg, BLH)
    res = bass_utils.run_bass_kernel_spmd(
        nc, in_maps, core_ids=list(range(cfg["R"])), trace=trace)
    out = np.concatenate([res.results[r]["out"] for r in range(cfg["R"])], axis=0)
    return out[: cfg["N"]], res


def kernel(**inputs) -> np.ndarray:
    out, _ = run(inputs, trace=False)
    return out.astype(np.float32)


# revision 17
# speedup vs baseline: 1.6894x; 1.3807x over previous
"""GAT (3-layer, PyG-style) Trainium2 Bass kernel, sharded across 8 NeuronCores.

Sharding: destination-node range partition (graph parallel). Per layer each
core computes h_ext = X_own @ [W | W.a_src | W.a_dst] for its nodes,
AllGathers h_ext (split into low/high halves so the collective overlaps
compute and gather indices fit int16), then aggregates all edges whose dst is
in its range: h_ext[src] rows come in via the dma_gather ucode path, edge
softmax weights are exp(leakyrelu(al_s+al_d)) (max-subtraction skipped -
mathematically identical, fp32-safe here), and the weighted scatter-add runs
as selection-matrix matmuls accumulating numerator + denominator in PSUM.
Self-loop edges are handled analytically from local rows (no gather).

kernel(**inputs) takes the FULL inputs and returns the FULL [N, 16] output.
"""

import sys

sys.path.insert(0, "/opt/trn_rl_repo")

import numpy as np

import concourse.bass as bass
import concourse.mybir as mybir
import concourse.tile as tile
from concourse import bacc
from concourse import bass_utils
from concourse.bass_interp import get_hw_module
from concourse.masks import make_identity
from concourse import library_config

F32 = mybir.dt.float32
BF = mybir.dt.bfloat16
I16 = mybir.dt.int16
import ml_dtypes
NPBF = ml_dtypes.bfloat16
P = 128


def real_cfg():
    R = 8
    N = 50000
    PER = N // R                      # 6250 nodes per core
    T = (PER + P - 1) // P            # 49 dst tiles per core
    return dict(
        R=R, N=N, PER=PER, T=T, NPAD=T * P,
        F_IN=128, HID=64, HEADS=8, N_CLASSES=16,
        NEG=0.2, SPLIT_T=25,
    )


# ---------------------------------------------------------------------------
# Host-side preprocessing
# ---------------------------------------------------------------------------

def _wrap16(flat):
    """int16 index list -> dma_gather idx layout [128, n/16]."""
    n = flat.shape[-1]
    w = flat.reshape(flat.shape[:-1] + (n // 16, 16))      # [..., c, 16]
    w = np.swapaxes(w, -1, -2)                             # [..., 16, c]
    reps = (1,) * (flat.ndim - 1) + (8, 1)
    return np.ascontiguousarray(np.tile(w, reps), np.int16)  # [..., 128, c]


def host_prepare(inputs, cfg):
    """Build per-core in_maps (numpy). Returns (in_maps, (B_LO, B_HI))."""
    R, N, PER, T, NPAD = cfg["R"], cfg["N"], cfg["PER"], cfg["T"], cfg["NPAD"]
    F_IN, HID, HEADS, NCLS = cfg["F_IN"], cfg["HID"], cfg["HEADS"], cfg["N_CLASSES"]
    HC = HID * HEADS
    SPLIT_T = cfg["SPLIT_T"]
    LO = SPLIT_T * P
    HI = NPAD - LO

    x = np.asarray(inputs["x"], np.float32)
    ei = np.asarray(inputs["edge_index"])
    src = ei[0].astype(np.int64)
    dst = ei[1].astype(np.int64)   # self-loops handled analytically on device

    core = dst // PER
    dloc = (dst - core * PER).astype(np.int64)
    sloc = (src % PER).astype(np.int64)
    srank = (src // PER).astype(np.int64)
    is_lo = sloc < LO
    tile_of = dloc // P

    # per (core, tile, group) counts -> global max block counts
    cl = np.zeros((R, T), np.int64)
    ch = np.zeros((R, T), np.int64)
    np.add.at(cl, (core[is_lo], tile_of[is_lo]), 1)
    np.add.at(ch, (core[~is_lo], tile_of[~is_lo]), 1)
    B_LO = int(np.ceil(cl.max() / P))
    B_HI = int(np.ceil(ch.max() / P))
    B = B_LO + B_HI

    idx_lo = np.zeros((R, T, B_LO * P), np.int16)
    idx_hi = np.zeros((R, T, B_HI * P), np.int16)
    dlc = np.full((R, T, P, B), -1.0, np.float32)  # cast to bf16 at the end

    # low-group gather row ids / high-group gather row ids
    grow = np.where(is_lo, srank * LO + sloc, srank * HI + (sloc - LO))

    # order edges by (core, tile, group, anything)
    order = np.lexsort((~is_lo * 1, tile_of, core))
    g_s = grow[order]
    d_s = dloc[order]
    core_s = core[order]
    tile_s = tile_of[order]
    lo_s = is_lo[order]

    grp = core_s * (2 * T) + tile_s * 2 + (~lo_s).astype(np.int64)
    grp_start = np.searchsorted(grp, np.arange(R * T * 2), side="left")
    pos = np.arange(len(grp)) - grp_start[grp]

    lo_m = lo_s
    hi_m = ~lo_s
    idx_lo[core_s[lo_m], tile_s[lo_m], pos[lo_m]] = g_s[lo_m].astype(np.int16)
    idx_hi[core_s[hi_m], tile_s[hi_m], pos[hi_m]] = g_s[hi_m].astype(np.int16)
    # flat position within the whole tile (lo blocks then hi blocks)
    fpos = np.where(lo_m, pos, B_LO * P + pos)
    dlc[core_s, tile_s, fpos % P, fpos // P] = (d_s - tile_s * P).astype(np.float32)
    dlc = dlc.astype(NPBF)
    # replicated row layout for S^T builds: [R, T, 128, B*128]
    dlcr = np.ascontiguousarray(np.broadcast_to(
        dlc.transpose(0, 1, 3, 2).reshape(R, T, 1, B * P), (R, T, P, B * P)))

    idx_lo = _wrap16(idx_lo)     # [R, T, 128, B_LO*8]
    idx_hi = _wrap16(idx_hi)

    # weight assembly: W'[f, :] = [W | W.a_src | W.a_dst | pad]
    def wext(W, a_s, a_d, ncols):
        Fin = W.shape[0]
        H, C = a_s.shape
        Wr = W.reshape(Fin, H, C)
        We = np.zeros((Fin, ncols), np.float32)
        We[:, : H * C] = W
        We[:, H * C : H * C + H] = np.einsum("fhc,hc->fh", Wr, a_s)
        We[:, H * C + H : H * C + 2 * H] = np.einsum("fhc,hc->fh", Wr, a_d)
        return We

    ROWG = 640
    ROWG2 = 128
    W0e = wext(np.asarray(inputs["W0"], np.float32),
               np.asarray(inputs["a_s0"], np.float32),
               np.asarray(inputs["a_d0"], np.float32), ROWG)
    W1e = wext(np.asarray(inputs["W1"], np.float32),
               np.asarray(inputs["a_s1"], np.float32),
               np.asarray(inputs["a_d1"], np.float32), ROWG)
    W2e = wext(np.asarray(inputs["W2"], np.float32),
               np.asarray(inputs["a_s2"], np.float32),
               np.asarray(inputs["a_d2"], np.float32), ROWG2)

    def bext(b, ncols):
        be = np.zeros((1, ncols), np.float32)
        be[0, : b.shape[0]] = b
        return np.ascontiguousarray(np.broadcast_to(be, (P, ncols)))

    b0e = bext(np.asarray(inputs["b0"], np.float32), ROWG)
    b1e = bext(np.asarray(inputs["b1"], np.float32), ROWG)
    b2e = bext(np.asarray(inputs["b2"], np.float32), ROWG2)

    W1e_r = W1e.reshape(4, P, ROWG).transpose(1, 0, 2).copy()
    W2e_r = W2e.reshape(4, P, ROWG2).transpose(1, 0, 2).copy()

    in_maps = []
    for r in range(R):
        xt0 = np.ascontiguousarray(x[r * PER : (r + 1) * PER].T)  # [F_IN, PER]
        in_maps.append({
            "xt0": xt0,
            "w0e": W0e, "w1e": W1e_r, "w2e": W2e_r,
            "b0e": b0e, "b1e": b1e, "b2e": b2e,
            "idx_lo": idx_lo[r], "idx_hi": idx_hi[r],
            "dlc": dlc[r], "dlcr": dlcr[r],
        })
    return in_maps, (B_LO, B_HI)


# ---------------------------------------------------------------------------
# Device program
# ---------------------------------------------------------------------------

def build_gat_nc(cfg, BLH):
    B_LO, B_HI = BLH
    B = B_LO + B_HI
    R, PER, T, NPAD = cfg["R"], cfg["PER"], cfg["T"], cfg["NPAD"]
    F_IN, HID, HEADS, NCLS = cfg["F_IN"], cfg["HID"], cfg["HEADS"], cfg["N_CLASSES"]
    NEG = cfg["NEG"]
    HC = HID * HEADS
    ROWG = 640
    ROWG2 = 128
    SPLIT_T = cfg["SPLIT_T"]
    LO = SPLIT_T * P
    HI = NPAD - LO

    nc = bacc.Bacc("TRN2", target_bir_lowering=False, debug=False,
                   num_devices=R)

    xt0_d = nc.dram_tensor("xt0", [F_IN, PER], F32, kind="ExternalInput")
    w0e_d = nc.dram_tensor("w0e", [F_IN, ROWG], F32, kind="ExternalInput")
    w1e_d = nc.dram_tensor("w1e", [P, 4, ROWG], F32, kind="ExternalInput")
    w2e_d = nc.dram_tensor("w2e", [P, 4, ROWG2], F32, kind="ExternalInput")
    b0e_d = nc.dram_tensor("b0e", [P, ROWG], F32, kind="ExternalInput")
    b1e_d = nc.dram_tensor("b1e", [P, ROWG], F32, kind="ExternalInput")
    b2e_d = nc.dram_tensor("b2e", [P, ROWG2], F32, kind="ExternalInput")
    ilo_d = nc.dram_tensor("idx_lo", [T, P, B_LO * 8], I16, kind="ExternalInput")
    ihi_d = nc.dram_tensor("idx_hi", [T, P, B_HI * 8], I16, kind="ExternalInput")
    dlcr_d = nc.dram_tensor("dlcr", [T, P, B * P], BF, kind="ExternalInput")
    dlc_d = nc.dram_tensor("dlc", [T, P, B], BF, kind="ExternalInput")
    out_d = nc.dram_tensor("out", [PER, NCLS], F32, kind="ExternalOutput")

    rg = [list(range(R))]

    with tile.TileContext(nc) as tc:
        with (
            tc.tile_pool(name="pers", bufs=1) as pers,
            tc.tile_pool(name="sb", bufs=2) as sb,
            tc.tile_pool(name="sbS", bufs=2 * B) as sbS,
            tc.tile_pool(name="ps", bufs=2, space="PSUM") as ps,
            tc.tile_pool(name="ps1", bufs=1, space="PSUM") as ps1,
            tc.tile_pool(name="dram", bufs=1, space="DRAM") as dram,
        ):
            nc.gpsimd.load_library(library_config.mlp)

            # ---- persistent tiles ----
            Xt = pers.tile([P, 4, NPAD], F32)          # feature-major X (own nodes)
            iota_i = pers.tile([P, P], I16)
            iota_row = pers.tile([P, P], BF)
            iota_col = pers.tile([P, P], BF)
            ident = pers.tile([P, P], F32)
            nc.gpsimd.iota(iota_i[:], pattern=[[1, P]], base=0, channel_multiplier=0)
            nc.vector.tensor_copy(iota_row[:], iota_i[:])
            nc.gpsimd.iota(iota_i[:], pattern=[[0, P]], base=0, channel_multiplier=1)
            nc.vector.tensor_copy(iota_col[:], iota_i[:])
            make_identity(nc, ident[:])

            w0_sb = pers.tile([P, 1, ROWG], F32)
            w1_sb = pers.tile([P, 4, ROWG], F32)
            w2_sb = pers.tile([P, 4, ROWG2], F32)
            b0_sb = pers.tile([P, ROWG], F32)
            b1_sb = pers.tile([P, ROWG], F32)
            b2_sb = pers.tile([P, ROWG2], F32)
            nc.sync.dma_start(w0_sb[:, 0, :], w0e_d[:, :])
            nc.sync.dma_start(w1_sb[:], w1e_d[:, :, :])
            nc.sync.dma_start(w2_sb[:], w2e_d[:, :, :])
            nc.sync.dma_start(b0_sb[:], b0e_d[:, :])
            nc.sync.dma_start(b1_sb[:], b1e_d[:, :])
            nc.sync.dma_start(b2_sb[:], b2e_d[:, :])

            if NPAD > PER:
                nc.vector.memset(Xt[:, 0, PER:NPAD], 0.0)
            nc.sync.dma_start(Xt[:, 0, :PER], xt0_d[:, :])

            # ---- internal DRAM ----
            hlo = [dram.tile([LO, ROWG], BF, name="hlo0"),
                   dram.tile([LO, ROWG], BF, name="hlo1"),
                   dram.tile([LO, ROWG2], BF, name="hlo2")]
            hhi = [dram.tile([HI, ROWG], BF, name="hhi0"),
                   dram.tile([HI, ROWG], BF, name="hhi1"),
                   dram.tile([HI, ROWG2], BF, name="hhi2")]
            tlo = [dram.tile([R * LO, ROWG], BF, addr_space="Shared", name="tlo0"),
                   dram.tile([R * LO, ROWG], BF, addr_space="Shared", name="tlo1"),
                   dram.tile([R * LO, ROWG2], BF, addr_space="Shared", name="tlo2")]
            thi = [dram.tile([R * HI, ROWG], BF, addr_space="Shared", name="thi0"),
                   dram.tile([R * HI, ROWG], BF, addr_space="Shared", name="thi1"),
                   dram.tile([R * HI, ROWG2], BF, addr_space="Shared", name="thi2")]

            for L in range(3):
                rowg = ROWG if L < 2 else ROWG2
                KC = 1 if L == 0 else 4
                nH = HEADS if L < 2 else 1
                ncols = HC if L < 2 else NCLS
                W_sb = [w0_sb, w1_sb, w2_sb][L]
                b_sb = [b0_sb, b1_sb, b2_sb][L]
                alow = ncols
                adoff = ncols + nH

                # ---------- h_ext = X_own @ W' + b' ----------
                for nt in range(T):
                    ph = ps1.tile([P, max(rowg, 528)], F32, tag="ph")
                    n1 = min(512, rowg)
                    for kc in range(KC):
                        nc.tensor.matmul(
                            ph[:, 0:n1],
                            lhsT=Xt[:, kc, nt * P : (nt + 1) * P],
                            rhs=W_sb[:, kc, 0:n1],
                            start=(kc == 0), stop=(kc == KC - 1),
                        )
                    if rowg > 512:
                        for kc in range(KC):
                            nc.tensor.matmul(
                                ph[:, 512:rowg],
                                lhsT=Xt[:, kc, nt * P : (nt + 1) * P],
                                rhs=W_sb[:, kc, 512:rowg],
                                start=(kc == 0), stop=(kc == KC - 1),
                            )
                    hsb = sb.tile([P, rowg], BF, tag="hsb")
                    nc.vector.tensor_tensor(hsb[:], ph[:, 0:rowg], b_sb[:],
                                            mybir.AluOpType.add)
                    if nt < SPLIT_T:
                        nc.sync.dma_start(hlo[L][nt * P : (nt + 1) * P, :], hsb[:])
                    else:
                        r0 = nt * P - LO
                        nc.sync.dma_start(hhi[L][r0 : r0 + P, :], hsb[:])
                    if nt == SPLIT_T - 1:
                        nc.gpsimd.collective_compute(
                            "AllGather", mybir.AluOpType.bypass,
                            replica_groups=rg, ins=[hlo[L][:, :]],
                            outs=[tlo[L][:, :]])
                nc.gpsimd.collective_compute(
                    "AllGather", mybir.AluOpType.bypass,
                    replica_groups=rg, ins=[hhi[L][:, :]],
                    outs=[thi[L][:, :]])

                # ---------- edge aggregation per dst tile ----------
                for t in range(T):
                    ilo = sb.tile([P, B_LO * 8], I16, tag="ilo")
                    ihi = sb.tile([P, B_HI * 8], I16, tag="ihi")
                    dlct = sb.tile([P, B], BF, tag="dlct")
                    dlrep = sb.tile([P, B * P], BF, tag="dlrep")
                    nc.sync.dma_start(ilo[:], ilo_d[t, :, :])
                    nc.sync.dma_start(ihi[:], ihi_d[t, :, :])
                    nc.sync.dma_start(dlct[:], dlc_d[t, :, :])
                    nc.sync.dma_start(dlrep[:], dlcr_d[t, :, :])

                    # local rows for the analytic self-loop term
                    loc = sb.tile([P, rowg], BF, tag="loc")
                    if t < SPLIT_T:
                        nc.sync.dma_start(loc[:], hlo[L][t * P : (t + 1) * P, :])
                    else:
                        r0 = t * P - LO
                        nc.sync.dma_start(loc[:], hhi[L][r0 : r0 + P, :])

                    g = sb.tile([P, B, rowg], BF, tag="g")
                    nc.gpsimd.dma_gather(
                        g[:, 0:B_LO, :], tlo[L][:, :], ilo[:],
                        num_idxs=B_LO * P, num_idxs_reg=B_LO * P,
                        elem_size=rowg)
                    nc.gpsimd.dma_gather(
                        g[:, B_LO:B, :], thi[L][:, :], ihi[:],
                        num_idxs=B_HI * P, num_idxs_reg=B_HI * P,
                        elem_size=rowg)
                    # al_d[dst] per edge: St = S^T via is_equal on the
                    # replicated dlc row; ad-MM contracts the dst axis.
                    psmall = ps.tile([P, B * nH + nH], F32, tag="psmall")
                    pad_ps = psmall[:, 0 : B * nH]
                    pd = psmall[:, B * nH : B * nH + nH]
                    for b in range(B):
                        St_b = sbS.tile([P, P], BF, tag="St")
                        nc.vector.tensor_tensor(
                            St_b[:], iota_col[:],
                            dlrep[:, b * P : (b + 1) * P],
                            mybir.AluOpType.is_equal)
                        nc.tensor.matmul(
                            pad_ps[:, b * nH : (b + 1) * nH],
                            lhsT=St_b[:], rhs=loc[:, adoff : adoff + nH],
                            start=True, stop=True)

                    # logits = al_s[src] + al_d[dst] ; lrelu ; exp
                    logits = sb.tile([P, B * nH], F32, tag="logits")
                    nc.vector.tensor_tensor(
                        logits[:].rearrange("p (b h) -> p b h", b=B),
                        g[:, :, alow : alow + nH],
                        pad_ps[:].rearrange("p (b h) -> p b h", b=B),
                        mybir.AluOpType.add)
                    lr = sb.tile([P, B * nH], F32, tag="lr")
                    nc.vector.tensor_scalar_mul(lr[:], logits[:], NEG)
                    nc.vector.tensor_tensor(lr[:], lr[:], logits[:],
                                            mybir.AluOpType.max)
                    w = sb.tile([P, B * nH], BF, tag="w")
                    nc.scalar.activation(w[:], lr[:],
                                         mybir.ActivationFunctionType.Exp)

                    # self-loop: ws = exp(lrelu(al_s_loc + al_d_loc))
                    sl = sb.tile([P, 3 * nH], F32, tag="sl")
                    nc.vector.tensor_tensor(sl[:, 0:nH],
                                            loc[:, alow : alow + nH],
                                            loc[:, adoff : adoff + nH],
                                            mybir.AluOpType.add)
                    nc.vector.tensor_scalar_mul(sl[:, nH : 2 * nH], sl[:, 0:nH], NEG)
                    nc.vector.tensor_tensor(sl[:, nH : 2 * nH], sl[:, nH : 2 * nH],
                                            sl[:, 0:nH], mybir.AluOpType.max)
                    ws = sb.tile([P, nH], F32, tag="ws")
                    nc.scalar.activation(ws[:], sl[:, nH : 2 * nH],
                                         mybir.ActivationFunctionType.Exp)

                    # weight gathered rows in place (al columns untouched)
                    nc.vector.tensor_tensor(
                        g[:, :, 0:ncols].rearrange("p b (h c) -> p b h c", h=nH),
                        g[:, :, 0:ncols].rearrange("p b (h c) -> p b h c", h=nH),
                        w[:].rearrange("p (b h) -> p b h", b=B)
                            .unsqueeze(3)
                            .to_broadcast([P, B, nH, ncols // nH]),
                        mybir.AluOpType.mult)

                    po = ps.tile([P, ncols], F32, tag="po")
                    for b in range(B):
                        S_b = sbS.tile([P, P], BF, tag="S")
                        nc.vector.tensor_tensor(
                            S_b[:],
                            dlct[:, b : b + 1].to_broadcast([P, P]),
                            iota_row[:],
                            mybir.AluOpType.is_equal)
                        nc.tensor.matmul(po[:], lhsT=S_b[:], rhs=g[:, b, 0:ncols],
                                         start=(b == 0), stop=(b == B - 1))
                        nc.tensor.matmul(pd[:], lhsT=S_b[:],
                                         rhs=w[:, b * nH : (b + 1) * nH],
                                         start=(b == 0), stop=(b == B - 1))

                    den = sb.tile([P, nH], F32, tag="den")
                    nc.vector.tensor_tensor(den[:], pd[:], ws[:],
                                            mybir.AluOpType.add)
                    rden = sb.tile([P, nH], F32, tag="rden")
                    nc.vector.reciprocal(rden[:], den[:])
                    wr = sb.tile([P, nH], F32, tag="wr")
                    nc.vector.tensor_tensor(wr[:], ws[:], rden[:],
                                            mybir.AluOpType.mult)

                    xn = sb.tile([P, ncols], F32, tag="xn")
                    nc.vector.tensor_tensor(
                        xn[:].rearrange("p (h c) -> p h c", h=nH),
                        po[:].rearrange("p (h c) -> p h c", h=nH),
                        rden[:].unsqueeze(2).to_broadcast([P, nH, ncols // nH]),
                        mybir.AluOpType.mult)
                    t2 = sb.tile([P, ncols], F32, tag="t2")
                    nc.vector.tensor_tensor(
                        t2[:].rearrange("p (h c) -> p h c", h=nH),
                        loc[:, 0:ncols].rearrange("p (h c) -> p h c", h=nH),
                        wr[:].unsqueeze(2).to_broadcast([P, nH, ncols // nH]),
                        mybir.AluOpType.mult)
                    nc.vector.tensor_tensor(xn[:], xn[:], t2[:],
                                            mybir.AluOpType.add)

                    if L < 2:
                        # ELU: xe = relu(x) + exp(min(x,0)) - 1
                        m = sb.tile([P, ncols], F32, tag="t2")
                        nc.vector.tensor_scalar_min(m[:], xn[:], 0.0)
                        em = sb.tile([P, ncols], F32, tag="t2")
                        nc.scalar.activation(em[:], m[:],
                                             mybir.ActivationFunctionType.Exp)
                        xe = sb.tile([P, ncols], F32, tag="xe")
                        nc.vector.tensor_scalar_max(xe[:], xn[:], 0.0)
                        nc.vector.tensor_tensor(xe[:], xe[:], em[:],
                                                mybir.AluOpType.add)
                        nc.vector.tensor_scalar_add(xe[:], xe[:], -1.0)
                        for c4 in range(ncols // P):
                            pt = ps.tile([P, P], F32, tag="pt")
                            nc.tensor.transpose(
                                pt[:], xe[:, c4 * P : (c4 + 1) * P], ident[:])
                            nc.scalar.copy(Xt[:, c4, t * P : (t + 1) * P], pt[:])
                    else:
                        rows = min(P, PER - t * P)
                        nc.sync.dma_start(out_d[t * P : t * P + rows, :],
                                          xn[:rows, 0:NCLS])

    nc.compile()
    nc.m = get_hw_module(nc.m)
    return nc


# ---------------------------------------------------------------------------
# Entry point
# ---------------------------------------------------------------------------

_CACHE = {}


def _get_nc(cfg, BLH):
    key = (tuple(sorted(cfg.items())), BLH)
    if key not in _CACHE:
        _CACHE[key] = build_gat_nc(cfg, BLH)
    return _CACHE[key]


def run(inputs, trace=False):
    cfg = real_cfg()
    in_maps, BLH = host_prepare(inputs, cfg)
    nc = _get_nc(cfg, BLH)
    res = bass_utils.run_bass_kernel_spmd(
        nc, in_maps, core_ids=list(range(cfg["R"])), trace=trace)
    out = np.concatenate([res.results[r]["out"] for r in range(cfg["R"])], axis=0)
    return out[: cfg["N"]], res


def kernel(**inputs) -> np.ndarray:
    out, _ = run(inputs, trace=False)
    return out.astype(np.float32)


# revision 18
# speedup vs baseline: 1.7064x; 1.0101x over previous
"""GAT (3-layer, PyG-style) Trainium2 Bass kernel, sharded across 8 NeuronCores.

Sharding: destination-node range partition (graph parallel). Per layer each
core computes h_ext = X_own @ [W | W.a_src | W.a_dst] for its nodes,
AllGathers h_ext (split into low/high halves so the collective overlaps
compute and gather indices fit int16), then aggregates all edges whose dst is
in its range: h_ext[src] rows come in via the dma_gather ucode path, edge
softmax weights are exp(leakyrelu(al_s+al_d)) (max-subtraction skipped -
mathematically identical, fp32-safe here), and the weighted scatter-add runs
as selection-matrix matmuls accumulating numerator + denominator in PSUM.
Self-loop edges are handled analytically from local rows (no gather).

kernel(**inputs) takes the FULL inputs and returns the FULL [N, 16] output.
"""

import sys

sys.path.insert(0, "/opt/trn_rl_repo")

import numpy as np

import concourse.bass as bass
import concourse.mybir as mybir
import concourse.tile as tile
from concourse import bacc
from concourse import bass_utils
from concourse.bass_interp import get_hw_module
from concourse.masks import make_identity
from concourse import library_config

F32 = mybir.dt.float32
BF = mybir.dt.bfloat16
I16 = mybir.dt.int16
import ml_dtypes
NPBF = ml_dtypes.bfloat16
P = 128


def real_cfg():
    R = 8
    N = 50000
    PER = N // R                      # 6250 nodes per core
    T = (PER + P - 1) // P            # 49 dst tiles per core
    return dict(
        R=R, N=N, PER=PER, T=T, NPAD=T * P,
        F_IN=128, HID=64, HEADS=8, N_CLASSES=16,
        NEG=0.2, SPLIT_T=25,
    )


# ---------------------------------------------------------------------------
# Host-side preprocessing
# ---------------------------------------------------------------------------

def _wrap16(flat):
    """int16 index list -> dma_gather idx layout [128, n/16]."""
    n = flat.shape[-1]
    w = flat.reshape(flat.shape[:-1] + (n // 16, 16))      # [..., c, 16]
    w = np.swapaxes(w, -1, -2)                             # [..., 16, c]
    reps = (1,) * (flat.ndim - 1) + (8, 1)
    return np.ascontiguousarray(np.tile(w, reps), np.int16)  # [..., 128, c]


def host_prepare(inputs, cfg):
    """Build per-core in_maps (numpy). Returns (in_maps, (B_LO, B_HI))."""
    R, N, PER, T, NPAD = cfg["R"], cfg["N"], cfg["PER"], cfg["T"], cfg["NPAD"]
    F_IN, HID, HEADS, NCLS = cfg["F_IN"], cfg["HID"], cfg["HEADS"], cfg["N_CLASSES"]
    HC = HID * HEADS
    SPLIT_T = cfg["SPLIT_T"]
    LO = SPLIT_T * P
    HI = NPAD - LO

    x = np.asarray(inputs["x"], np.float32)
    ei = np.asarray(inputs["edge_index"])
    src = ei[0].astype(np.int64)
    dst = ei[1].astype(np.int64)   # self-loops handled analytically on device

    core = dst // PER
    dloc = (dst - core * PER).astype(np.int64)
    sloc = (src % PER).astype(np.int64)
    srank = (src // PER).astype(np.int64)
    is_lo = sloc < LO
    tile_of = dloc // P

    # per (core, tile, group) counts -> global max block counts
    cl = np.zeros((R, T), np.int64)
    ch = np.zeros((R, T), np.int64)
    np.add.at(cl, (core[is_lo], tile_of[is_lo]), 1)
    np.add.at(ch, (core[~is_lo], tile_of[~is_lo]), 1)
    B_LO = int(np.ceil(cl.max() / P))
    B_HI = int(np.ceil(ch.max() / P))
    B = B_LO + B_HI

    idx_lo = np.zeros((R, T, B_LO * P), np.int16)
    idx_hi = np.zeros((R, T, B_HI * P), np.int16)
    dlc = np.full((R, T, P, B), -1.0, np.float32)  # cast to bf16 at the end

    # low-group gather row ids / high-group gather row ids
    grow = np.where(is_lo, srank * LO + sloc, srank * HI + (sloc - LO))

    # order edges by (core, tile, group, anything)
    order = np.lexsort((~is_lo * 1, tile_of, core))
    g_s = grow[order]
    d_s = dloc[order]
    core_s = core[order]
    tile_s = tile_of[order]
    lo_s = is_lo[order]

    grp = core_s * (2 * T) + tile_s * 2 + (~lo_s).astype(np.int64)
    grp_start = np.searchsorted(grp, np.arange(R * T * 2), side="left")
    pos = np.arange(len(grp)) - grp_start[grp]

    lo_m = lo_s
    hi_m = ~lo_s
    idx_lo[core_s[lo_m], tile_s[lo_m], pos[lo_m]] = g_s[lo_m].astype(np.int16)
    idx_hi[core_s[hi_m], tile_s[hi_m], pos[hi_m]] = g_s[hi_m].astype(np.int16)
    # flat position within the whole tile (lo blocks then hi blocks)
    fpos = np.where(lo_m, pos, B_LO * P + pos)
    dlc[core_s, tile_s, fpos % P, fpos // P] = (d_s - tile_s * P).astype(np.float32)
    dlc = dlc.astype(NPBF)
    # replicated row layout for S^T builds: [R, T, 128, B*128]
    dlcr = np.ascontiguousarray(np.broadcast_to(
        dlc.transpose(0, 1, 3, 2).reshape(R, T, 1, B * P), (R, T, P, B * P)))

    idx_lo = _wrap16(idx_lo)     # [R, T, 128, B_LO*8]
    idx_hi = _wrap16(idx_hi)

    # weight assembly: W'[f, :] = [W | W.a_src | W.a_dst | pad]
    def wext(W, a_s, a_d, ncols):
        Fin = W.shape[0]
        H, C = a_s.shape
        Wr = W.reshape(Fin, H, C)
        We = np.zeros((Fin, ncols), np.float32)
        We[:, : H * C] = W
        We[:, H * C : H * C + H] = np.einsum("fhc,hc->fh", Wr, a_s)
        We[:, H * C + H : H * C + 2 * H] = np.einsum("fhc,hc->fh", Wr, a_d)
        return We

    ROWG = 640
    ROWG2 = 128
    W0e = wext(np.asarray(inputs["W0"], np.float32),
               np.asarray(inputs["a_s0"], np.float32),
               np.asarray(inputs["a_d0"], np.float32), ROWG)
    W1e = wext(np.asarray(inputs["W1"], np.float32),
               np.asarray(inputs["a_s1"], np.float32),
               np.asarray(inputs["a_d1"], np.float32), ROWG)
    W2e = wext(np.asarray(inputs["W2"], np.float32),
               np.asarray(inputs["a_s2"], np.float32),
               np.asarray(inputs["a_d2"], np.float32), ROWG2)

    def bext(b, ncols):
        be = np.zeros((1, ncols), np.float32)
        be[0, : b.shape[0]] = b
        return np.ascontiguousarray(np.broadcast_to(be, (P, ncols)))

    b0e = bext(np.asarray(inputs["b0"], np.float32), ROWG)
    b1e = bext(np.asarray(inputs["b1"], np.float32), ROWG)
    b2e = bext(np.asarray(inputs["b2"], np.float32), ROWG2)

    W1e_r = W1e.reshape(4, P, ROWG).transpose(1, 0, 2).copy()
    W2e_r = W2e.reshape(4, P, ROWG2).transpose(1, 0, 2).copy()

    in_maps = []
    for r in range(R):
        xt0 = np.ascontiguousarray(x[r * PER : (r + 1) * PER].T)  # [F_IN, PER]
        in_maps.append({
            "xt0": xt0,
            "w0e": W0e, "w1e": W1e_r, "w2e": W2e_r,
            "b0e": b0e, "b1e": b1e, "b2e": b2e,
            "idx_lo": idx_lo[r], "idx_hi": idx_hi[r],
            "dlc": dlc[r], "dlcr": dlcr[r],
        })
    return in_maps, (B_LO, B_HI)


# ---------------------------------------------------------------------------
# Device program
# ---------------------------------------------------------------------------

def build_gat_nc(cfg, BLH):
    B_LO, B_HI = BLH
    B = B_LO + B_HI
    R, PER, T, NPAD = cfg["R"], cfg["PER"], cfg["T"], cfg["NPAD"]
    F_IN, HID, HEADS, NCLS = cfg["F_IN"], cfg["HID"], cfg["HEADS"], cfg["N_CLASSES"]
    NEG = cfg["NEG"]
    HC = HID * HEADS
    ROWG = 640
    ROWG2 = 128
    SPLIT_T = cfg["SPLIT_T"]
    LO = SPLIT_T * P
    HI = NPAD - LO

    nc = bacc.Bacc("TRN2", target_bir_lowering=False, debug=False,
                   num_devices=R)

    xt0_d = nc.dram_tensor("xt0", [F_IN, PER], F32, kind="ExternalInput")
    w0e_d = nc.dram_tensor("w0e", [F_IN, ROWG], F32, kind="ExternalInput")
    w1e_d = nc.dram_tensor("w1e", [P, 4, ROWG], F32, kind="ExternalInput")
    w2e_d = nc.dram_tensor("w2e", [P, 4, ROWG2], F32, kind="ExternalInput")
    b0e_d = nc.dram_tensor("b0e", [P, ROWG], F32, kind="ExternalInput")
    b1e_d = nc.dram_tensor("b1e", [P, ROWG], F32, kind="ExternalInput")
    b2e_d = nc.dram_tensor("b2e", [P, ROWG2], F32, kind="ExternalInput")
    ilo_d = nc.dram_tensor("idx_lo", [T, P, B_LO * 8], I16, kind="ExternalInput")
    ihi_d = nc.dram_tensor("idx_hi", [T, P, B_HI * 8], I16, kind="ExternalInput")
    dlcr_d = nc.dram_tensor("dlcr", [T, P, B * P], BF, kind="ExternalInput")
    dlc_d = nc.dram_tensor("dlc", [T, P, B], BF, kind="ExternalInput")
    out_d = nc.dram_tensor("out", [PER, NCLS], F32, kind="ExternalOutput")

    rg = [list(range(R))]

    with tile.TileContext(nc) as tc:
        with (
            tc.tile_pool(name="pers", bufs=1) as pers,
            tc.tile_pool(name="sb", bufs=2) as sb,
            tc.tile_pool(name="sbS", bufs=2 * B) as sbS,
            tc.tile_pool(name="ps", bufs=2, space="PSUM") as ps,
            tc.tile_pool(name="ps1", bufs=1, space="PSUM") as ps1,
            tc.tile_pool(name="dram", bufs=1, space="DRAM") as dram,
        ):
            nc.gpsimd.load_library(library_config.mlp)

            # ---- persistent tiles ----
            Xt = pers.tile([P, 4, NPAD], F32)          # feature-major X (own nodes)
            iota_i = pers.tile([P, P], I16)
            iota_row = pers.tile([P, P], BF)
            iota_col = pers.tile([P, P], BF)
            ident = pers.tile([P, P], F32)
            nc.gpsimd.iota(iota_i[:], pattern=[[1, P]], base=0, channel_multiplier=0)
            nc.vector.tensor_copy(iota_row[:], iota_i[:])
            nc.gpsimd.iota(iota_i[:], pattern=[[0, P]], base=0, channel_multiplier=1)
            nc.vector.tensor_copy(iota_col[:], iota_i[:])
            make_identity(nc, ident[:])

            w0_sb = pers.tile([P, 1, ROWG], F32)
            w1_sb = pers.tile([P, 4, ROWG], F32)
            w2_sb = pers.tile([P, 4, ROWG2], F32)
            b0_sb = pers.tile([P, ROWG], F32)
            b1_sb = pers.tile([P, ROWG], F32)
            b2_sb = pers.tile([P, ROWG2], F32)
            nc.sync.dma_start(w0_sb[:, 0, :], w0e_d[:, :])
            nc.sync.dma_start(w1_sb[:], w1e_d[:, :, :])
            nc.sync.dma_start(w2_sb[:], w2e_d[:, :, :])
            nc.sync.dma_start(b0_sb[:], b0e_d[:, :])
            nc.sync.dma_start(b1_sb[:], b1e_d[:, :])
            nc.sync.dma_start(b2_sb[:], b2e_d[:, :])

            if NPAD > PER:
                nc.vector.memset(Xt[:, 0, PER:NPAD], 0.0)
            nc.sync.dma_start(Xt[:, 0, :PER], xt0_d[:, :])

            # ---- internal DRAM ----
            hlo = [dram.tile([LO, ROWG], BF, name="hlo0"),
                   dram.tile([LO, ROWG], BF, name="hlo1"),
                   dram.tile([LO, ROWG2], BF, name="hlo2")]
            hhi = [dram.tile([HI, ROWG], BF, name="hhi0"),
                   dram.tile([HI, ROWG], BF, name="hhi1"),
                   dram.tile([HI, ROWG2], BF, name="hhi2")]
            tlo = [dram.tile([R * LO, ROWG], BF, addr_space="Shared", name="tlo0"),
                   dram.tile([R * LO, ROWG], BF, addr_space="Shared", name="tlo1"),
                   dram.tile([R * LO, ROWG2], BF, addr_space="Shared", name="tlo2")]
            thi = [dram.tile([R * HI, ROWG], BF, addr_space="Shared", name="thi0"),
                   dram.tile([R * HI, ROWG], BF, addr_space="Shared", name="thi1"),
                   dram.tile([R * HI, ROWG2], BF, addr_space="Shared", name="thi2")]

            for L in range(3):
                rowg = ROWG if L < 2 else ROWG2
                KC = 1 if L == 0 else 4
                nH = HEADS if L < 2 else 1
                ncols = HC if L < 2 else NCLS
                W_sb = [w0_sb, w1_sb, w2_sb][L]
                b_sb = [b0_sb, b1_sb, b2_sb][L]
                alow = ncols
                adoff = ncols + nH

                # ---------- h_ext = X_own @ W' + b' ----------
                for nt in range(T):
                    ph = ps1.tile([P, max(rowg, 528)], F32, tag="ph")
                    n1 = min(512, rowg)
                    for kc in range(KC):
                        nc.tensor.matmul(
                            ph[:, 0:n1],
                            lhsT=Xt[:, kc, nt * P : (nt + 1) * P],
                            rhs=W_sb[:, kc, 0:n1],
                            start=(kc == 0), stop=(kc == KC - 1),
                        )
                    if rowg > 512:
                        for kc in range(KC):
                            nc.tensor.matmul(
                                ph[:, 512:rowg],
                                lhsT=Xt[:, kc, nt * P : (nt + 1) * P],
                                rhs=W_sb[:, kc, 512:rowg],
                                start=(kc == 0), stop=(kc == KC - 1),
                            )
                    hsb = sb.tile([P, rowg], BF, tag="hsb")
                    nc.vector.tensor_tensor(hsb[:], ph[:, 0:rowg], b_sb[:],
                                            mybir.AluOpType.add)
                    if nt < SPLIT_T:
                        nc.sync.dma_start(hlo[L][nt * P : (nt + 1) * P, :], hsb[:])
                    else:
                        r0 = nt * P - LO
                        nc.sync.dma_start(hhi[L][r0 : r0 + P, :], hsb[:])
                    if nt == SPLIT_T - 1:
                        nc.gpsimd.collective_compute(
                            "AllGather", mybir.AluOpType.bypass,
                            replica_groups=rg, ins=[hlo[L][:, :]],
                            outs=[tlo[L][:, :]])
                nc.gpsimd.collective_compute(
                    "AllGather", mybir.AluOpType.bypass,
                    replica_groups=rg, ins=[hhi[L][:, :]],
                    outs=[thi[L][:, :]])

                # ---------- edge aggregation per dst tile ----------
                for t in range(T):
                    ilo = sb.tile([P, B_LO * 8], I16, tag="ilo")
                    ihi = sb.tile([P, B_HI * 8], I16, tag="ihi")
                    dlct = sb.tile([P, B], BF, tag="dlct")
                    dlrep = sb.tile([P, B * P], BF, tag="dlrep")
                    nc.sync.dma_start(ilo[:], ilo_d[t, :, :])
                    nc.sync.dma_start(ihi[:], ihi_d[t, :, :])
                    nc.sync.dma_start(dlct[:], dlc_d[t, :, :])
                    nc.sync.dma_start(dlrep[:], dlcr_d[t, :, :])

                    # local rows for the analytic self-loop term
                    loc = sb.tile([P, rowg], BF, tag="loc")
                    if t < SPLIT_T:
                        nc.sync.dma_start(loc[:], hlo[L][t * P : (t + 1) * P, :])
                    else:
                        r0 = t * P - LO
                        nc.sync.dma_start(loc[:], hhi[L][r0 : r0 + P, :])

                    g = sb.tile([P, B, rowg], BF, tag="g")
                    nc.gpsimd.dma_gather(
                        g[:, 0:B_LO, :], tlo[L][:, :], ilo[:],
                        num_idxs=B_LO * P, num_idxs_reg=B_LO * P,
                        elem_size=rowg)
                    nc.gpsimd.dma_gather(
                        g[:, B_LO:B, :], thi[L][:, :], ihi[:],
                        num_idxs=B_HI * P, num_idxs_reg=B_HI * P,
                        elem_size=rowg)
                    # al_d[dst] per edge: St = S^T via is_equal on the
                    # replicated dlc row; ad-MM contracts the dst axis.
                    psmall = ps.tile([P, B * nH + nH], F32, tag="psmall")
                    pad_ps = psmall[:, 0 : B * nH]
                    pd = psmall[:, B * nH : B * nH + nH]
                    for b in range(B):
                        St_b = sbS.tile([P, P], BF, tag="St")
                        nc.vector.tensor_tensor(
                            St_b[:], iota_col[:],
                            dlrep[:, b * P : (b + 1) * P],
                            mybir.AluOpType.is_equal)
                        nc.tensor.matmul(
                            pad_ps[:, b * nH : (b + 1) * nH],
                            lhsT=St_b[:], rhs=loc[:, adoff : adoff + nH],
                            start=True, stop=True)

                    # logits = al_s[src] + al_d[dst] ; lrelu ; exp
                    logits = sb.tile([P, B * nH], F32, tag="logits")
                    nc.vector.tensor_tensor(
                        logits[:].rearrange("p (b h) -> p b h", b=B),
                        g[:, :, alow : alow + nH],
                        pad_ps[:].rearrange("p (b h) -> p b h", b=B),
                        mybir.AluOpType.add)
                    lr = sb.tile([P, B * nH], F32, tag="lr")
                    nc.vector.tensor_scalar_mul(lr[:], logits[:], NEG)
                    nc.vector.tensor_tensor(lr[:], lr[:], logits[:],
                                            mybir.AluOpType.max)
                    w = sb.tile([P, B * nH], BF, tag="w")
                    nc.scalar.activation(w[:], lr[:],
                                         mybir.ActivationFunctionType.Exp)

                    # self-loop: ws = exp(lrelu(al_s_loc + al_d_loc))
                    sl = sb.tile([P, 3 * nH], F32, tag="sl")
                    nc.vector.tensor_tensor(sl[:, 0:nH],
                                            loc[:, alow : alow + nH],
                                            loc[:, adoff : adoff + nH],
                                            mybir.AluOpType.add)
                    nc.vector.tensor_scalar_mul(sl[:, nH : 2 * nH], sl[:, 0:nH], NEG)
                    nc.vector.tensor_tensor(sl[:, nH : 2 * nH], sl[:, nH : 2 * nH],
                                            sl[:, 0:nH], mybir.AluOpType.max)
                    ws = sb.tile([P, nH], F32, tag="ws")
                    nc.scalar.activation(ws[:], sl[:, nH : 2 * nH],
                                         mybir.ActivationFunctionType.Exp)

                    # weighted gathered rows -> contiguous tile
                    gw = sb.tile([P, B, ncols], BF, tag="gw")
                    nc.vector.tensor_tensor(
                        gw[:],
                        g[:, :, 0:ncols].rearrange("p b (h c) -> p b h c", h=nH),
                        w[:].rearrange("p (b h) -> p b h", b=B)
                            .unsqueeze(3)
                            .to_broadcast([P, B, nH, ncols // nH]),
                        mybir.AluOpType.mult)

                    po = ps.tile([P, ncols], F32, tag="po")
                    for b in range(B):
                        S_b = sbS.tile([P, P], BF, tag="S")
                        nc.vector.tensor_tensor(
                            S_b[:],
                            dlct[:, b : b + 1].to_broadcast([P, P]),
                            iota_row[:],
                            mybir.AluOpType.is_equal)
                        nc.tensor.matmul(po[:], lhsT=S_b[:], rhs=gw[:, b, :],
                                         start=(b == 0), stop=(b == B - 1))
                        nc.tensor.matmul(pd[:], lhsT=S_b[:],
                                         rhs=w[:, b * nH : (b + 1) * nH],
                                         start=(b == 0), stop=(b == B - 1))

                    den = sb.tile([P, nH], F32, tag="den")
                    nc.vector.tensor_tensor(den[:], pd[:], ws[:],
                                            mybir.AluOpType.add)
                    rden = sb.tile([P, nH], F32, tag="rden")
                    nc.vector.reciprocal(rden[:], den[:])
                    wr = sb.tile([P, nH], F32, tag="wr")
                    nc.vector.tensor_tensor(wr[:], ws[:], rden[:],
                                            mybir.AluOpType.mult)

                    xn = sb.tile([P, ncols], F32, tag="xn")
                    nc.vector.tensor_tensor(
                        xn[:].rearrange("p (h c) -> p h c", h=nH),
                        po[:].rearrange("p (h c) -> p h c", h=nH),
                        rden[:].unsqueeze(2).to_broadcast([P, nH, ncols // nH]),
                        mybir.AluOpType.mult)
                    t2 = sb.tile([P, ncols], F32, tag="t2")
                    nc.vector.tensor_tensor(
                        t2[:].rearrange("p (h c) -> p h c", h=nH),
                        loc[:, 0:ncols].rearrange("p (h c) -> p h c", h=nH),
                        wr[:].unsqueeze(2).to_broadcast([P, nH, ncols // nH]),
                        mybir.AluOpType.mult)
                    nc.vector.tensor_tensor(xn[:], xn[:], t2[:],
                                            mybir.AluOpType.add)

                    if L < 2:
                        # ELU: xe = relu(x) + exp(-relu(-x)) - 1  (ACT-heavy)
                        m = sb.tile([P, ncols], F32, tag="t2")
                        nc.scalar.activation(m[:], xn[:],
                                             mybir.ActivationFunctionType.Relu,
                                             scale=-1.0)
                        em = sb.tile([P, ncols], F32, tag="t2")
                        nc.scalar.activation(em[:], m[:],
                                             mybir.ActivationFunctionType.Exp,
                                             scale=-1.0)
                        xe = sb.tile([P, ncols], F32, tag="xe")
                        nc.scalar.activation(xe[:], xn[:],
                                             mybir.ActivationFunctionType.Relu)
                        nc.vector.tensor_tensor(xe[:], xe[:], em[:],
                                                mybir.AluOpType.add)
                        nc.vector.tensor_scalar_add(xe[:], xe[:], -1.0)
                        for c4 in range(ncols // P):
                            pt = ps.tile([P, P], F32, tag="pt")
                            nc.tensor.transpose(
                                pt[:], xe[:, c4 * P : (c4 + 1) * P], ident[:])
                            nc.scalar.copy(Xt[:, c4, t * P : (t + 1) * P], pt[:])
                    else:
                        rows = min(P, PER - t * P)
                        nc.sync.dma_start(out_d[t * P : t * P + rows, :],
                                          xn[:rows, 0:NCLS])

    nc.compile()
    nc.m = get_hw_module(nc.m)
    return nc


# ---------------------------------------------------------------------------
# Entry point
# ---------------------------------------------------------------------------

_CACHE = {}


def _get_nc(cfg, BLH):
    key = (tuple(sorted(cfg.items())), BLH)
    if key not in _CACHE:
        _CACHE[key] = build_gat_nc(cfg, BLH)
    return _CACHE[key]


def run(inputs, trace=False):
    cfg = real_cfg()
    in_maps, BLH = host_prepare(inputs, cfg)
    nc = _get_nc(cfg, BLH)
    res = bass_utils.run_bass_kernel_spmd(
        nc, in_maps, core_ids=list(range(cfg["R"])), trace=trace)
    out = np.concatenate([res.results[r]["out"] for r in range(cfg["R"])], axis=0)
    return out[: cfg["N"]], res


def kernel(**inputs) -> np.ndarray:
    out, _ = run(inputs, trace=False)
    return out.astype(np.float32)
